# revision 6
# baseline (speedup 1.0000x reference)
"""ChannelAttention TRN2 Bass kernel.

Reference computation (per batch b):
    qkv = x @ Wqkv -> q,k,v  [N, G*hd] each
    S_g = (k_g * scale)^T @ v_g          [hd, hd] per group
    attn_g = softmax(S_g, axis=-1)
    out[n, (g,d)] = sum_e attn_g[d,e] * q[n, (g,e)]
    y = out @ Wproj + bproj
    out = LayerNorm(x + y) * gamma + beta

Key identity used: attention is shared across all tokens n, so
    y = x @ M,   M = Wq' @ Wproj,  Wq'[c,(g,d)] = sum_e Wq[c,(g,e)] attn_g[d,e]
and S_g = (Wk_g*scale)^T @ X2 @ Wv_g with X2 = x^T x (Gram matrix).
The residual is folded in as h = x @ (M + I); LayerNorm runs off PSUM.

Sharding: data-parallel over B; core b computes batch b. No collectives.
All heavy matmuls run in float32r (~13-bit mantissa, 4x faster than fp32).
"""

import numpy as np
from contextlib import ExitStack

import concourse.bass as bass
import concourse.tile as tile
from concourse import mybir, bacc
from concourse.bass_utils import run_bass_kernel_spmd
from concourse.masks import make_identity

B, N, C = 8, 4096, 1024
G, HD = 16, C // 16
SCALE = HD ** -0.5
P = 128
NT = N // P          # 32 row tiles of 128
CCH = C // P         # 8 column chunks of 128
NPAIR = CCH          # 8 group pairs (2 groups of 64 cols = 128)
LN_EPS = 1e-5
F32 = mybir.dt.float32
F32R = mybir.dt.float32r

_cache = {}


def _build(need_bias, need_gamma, need_beta):
    nc = bacc.Bacc(None, target_bir_lowering=False)

    x_in = nc.declare_dram_parameter("x", [N, C], F32R, isOutput=False)
    xt_in = nc.declare_dram_parameter("xt", [C, N], F32R, isOutput=False)
    wv_in = nc.declare_dram_parameter("wv", [C, C], F32R, isOutput=False)
    wk_in = nc.declare_dram_parameter("wk", [C, C], F32R, isOutput=False)   # pre-scaled
    wqt_in = nc.declare_dram_parameter("wqt", [C, C], F32R, isOutput=False)  # Wq^T
    wp_in = nc.declare_dram_parameter("wp", [C, C], F32R, isOutput=False)
    if need_bias:
        bias_in = nc.declare_dram_parameter("bias", [1, C], F32R, isOutput=False)
    if need_gamma:
        gamma_in = nc.declare_dram_parameter("gamma", [1, C], F32, isOutput=False)
    if need_beta:
        beta_in = nc.declare_dram_parameter("beta", [1, C], F32, isOutput=False)
    out_d = nc.declare_dram_parameter("out", [N, C], F32, isOutput=True)

    with tile.TileContext(nc) as tc, ExitStack() as octx:
        def copy_psum(out, in_, use_act):
            if use_act:
                nc.scalar.activation(
                    out=out, in_=in_,
                    func=mybir.ActivationFunctionType.Copy,
                    bias=0.0, scale=1.0)
            else:
                nc.vector.tensor_copy(out=out, in_=in_)

        consts = octx.enter_context(tc.tile_pool(name="consts", bufs=1))
        eps_sb = consts.tile([P, 1], F32)
        nc.vector.memset(eps_sb, LN_EPS)
        ident_tr = consts.tile([P, P], F32)  # for PE transposes of exp blocks
        make_identity(nc, ident_tr)
        ident_r = consts.tile([P, P], F32)  # identity added to M diagonal
        make_identity(nc, ident_r)
        if need_bias:
            ones_r = consts.tile([1, P], F32R)
            nc.vector.memset(ones_r, 1.0)
            bias_sb = consts.tile([1, C], F32R)
            nc.sync.dma_start(out=bias_sb, in_=bias_in[:, :])
        if need_gamma:
            gamma_sb = consts.tile([P, C], F32)
            g_ap = gamma_in[:, :]
            nc.gpsimd.dma_start(
                out=gamma_sb,
                in_=bass.AP(tensor=g_ap.tensor, offset=g_ap.offset,
                            ap=[[0, P], g_ap.ap[-1]]),
            )
        if need_beta:
            beta_sb = consts.tile([P, C], F32)
            b_ap = beta_in[:, :]
            nc.gpsimd.dma_start(
                out=beta_sb,
                in_=bass.AP(tensor=b_ap.tensor, offset=b_ap.offset,
                            ap=[[0, P], b_ap.ap[-1]]),
            )

        # attn pipeline results, alive from phase A into phase B
        attn_pool = octx.enter_context(tc.tile_pool(name="attn", bufs=1))
        blockdiag = []
        for i in range(NPAIR):
            bd = attn_pool.tile([P, P], F32R, tag=f"bd{i}", name=f"bd{i}")
            blockdiag.append(bd)
        recip = attn_pool.tile([P, NPAIR], F32)  # 1/rowsum, [d-in-pair, pair]

        # ---------------- Phase A: X2 = x^T x ----------------
        with tc.tile_pool(name="x2sb", bufs=1) as x2pool:
            X2_sb = x2pool.tile([P, CCH, C], F32R)
            with tc.tile_pool(name="xres", bufs=1) as xres:
                xs = []
                for i in range(8):
                    xt_tile = xres.tile([P, 4, C], F32R, tag=f"x{i}", name=f"x{i}")
                    nc.sync.dma_start(
                        out=xt_tile,
                        in_=x_in[i * 512:(i + 1) * 512, :].rearrange(
                            "(tt p) c -> p tt c", p=P),
                    )
                    xs.append(xt_tile)

                for half in range(2):  # c-chunks [0..3] then [4..7]
                    with tc.tile_pool(name="x2ps", bufs=1, space="PSUM") as x2ps:
                        ps_tiles = []
                        for q in range(4):
                            x2pt = x2ps.tile([P, C], F32, tag=f"x2p{q}")
                            ps_tiles.append(x2pt)
                        for t in range(NT):
                            xtile = xs[t // 4][:, t % 4, :]
                            for q in range(4):
                                cch = half * 4 + q
                                for j in range(2):
                                    nc.tensor.matmul(
                                        out=ps_tiles[q][:, j * 512:(j + 1) * 512],
                                        lhsT=xtile[:, cch * P:(cch + 1) * P],
                                        rhs=xtile[:, j * 512:(j + 1) * 512],
                                        start=(t == 0), stop=(t == NT - 1),
                                    )
                        for q in range(4):
                            cch = half * 4 + q
                            copy_psum(X2_sb[:, cch, :], ps_tiles[q], q % 2 == 1)

            # ---------------- T = X2 @ Wv ----------------
            with tc.tile_pool(name="wvp", bufs=1) as wvp, \
                 tc.tile_pool(name="tsb", bufs=1) as tpool, \
                 tc.tile_pool(name="tps", bufs=2, space="PSUM") as tps:
                Wv_sb = wvp.tile([P, CCH, C], F32R)
                nc.sync.dma_start(
                    out=Wv_sb,
                    in_=wv_in[:, :].rearrange("(cc p) j -> p cc j", p=P))
                T_sb = tpool.tile([P, CCH, C], F32R)
                for cch in range(CCH):
                    ps_t = tps.tile([P, C], F32, tag="tp")
                    for jch in range(2):
                        for k in range(CCH):
                            nc.tensor.matmul(
                                out=ps_t[:, jch * 512:(jch + 1) * 512],
                                lhsT=X2_sb[:, k, cch * P:(cch + 1) * P],
                                rhs=Wv_sb[:, k, jch * 512:(jch + 1) * 512],
                                start=(k == 0), stop=(k == CCH - 1),
                            )
                    copy_psum(T_sb[:, cch, :], ps_t, cch % 2 == 1)

                # ---------------- S = Wk_sc^T @ T (pair-packed) + softmax ----
                with tc.tile_pool(name="wkp", bufs=1) as wkp, \
                     tc.tile_pool(name="sps", bufs=1, space="PSUM") as sps, \
                     tc.tile_pool(name="smax", bufs=2) as smax, \
                     tc.tile_pool(name="trps", bufs=2, space="PSUM") as trps:
                    Wk_sb = wkp.tile([P, CCH, C], F32R)
                    nc.sync.dma_start(
                        out=Wk_sb,
                        in_=wk_in[:, :].rearrange("(cc p) j -> p cc j", p=P))
                    ps_s = sps.tile([P, NPAIR * P], F32)
                    for i in range(NPAIR):
                        for k in range(CCH):
                            nc.tensor.matmul(
                                out=ps_s[:, i * P:(i + 1) * P],
                                lhsT=Wk_sb[:, k, i * P:(i + 1) * P],
                                rhs=T_sb[:, k, i * P:(i + 1) * P],
                                start=(k == 0), stop=(k == CCH - 1),
                            )
                    for i in range(NPAIR):
                        # exp(S - rowmax) of the two diagonal 64x64 blocks,
                        # written into a zeroed [128,128] tile; transposing the
                        # whole tile then yields blockdiag(attnT_g0, attnT_g1).
                        negmax = smax.tile([P, 1], F32, tag="negmax")
                        exp_full = smax.tile([P, P], F32, tag="exp")
                        nc.vector.memset(exp_full, 0.0)
                        sums = smax.tile([P, 1], F32, tag="sums")
                        for h in range(2):
                            pr = slice(h * HD, (h + 1) * HD)
                            blk = ps_s[pr, i * P + h * HD: i * P + (h + 1) * HD]
                            nc.vector.reduce_max(out=negmax[pr], in_=blk,
                                                 axis=mybir.AxisListType.X)
                            nc.scalar.mul(out=negmax[pr], in_=negmax[pr],
                                          mul=-1.0)
                            nc.scalar.activation(
                                out=exp_full[pr, h * HD:(h + 1) * HD], in_=blk,
                                func=mybir.ActivationFunctionType.Exp,
                                bias=negmax[pr], scale=1.0)
                            nc.vector.reduce_sum(
                                out=sums[pr],
                                in_=exp_full[pr, h * HD:(h + 1) * HD],
                                axis=mybir.AxisListType.X)
                        nc.vector.reciprocal(out=recip[:, i:i + 1], in_=sums)
                        ps_tr = trps.tile([P, P], F32, tag="tr")
                        nc.tensor.transpose(out=ps_tr, in_=exp_full,
                                            identity=ident_tr)
                        nc.vector.tensor_copy(out=blockdiag[i], in_=ps_tr)

        # ---------------- Phase B: Wq'^T, M = Wq' Wproj + I ----------------
        with tc.tile_pool(name="msb", bufs=1) as mpool:
            Msb = mpool.tile([P, CCH, C], F32R)
            with tc.tile_pool(name="wqtp", bufs=1) as wqtp, \
                 tc.tile_pool(name="wqp", bufs=1) as wqp, \
                 tc.tile_pool(name="wqps", bufs=3, space="PSUM") as wqps:
                WqT_sb = wqtp.tile([P, NPAIR, C], F32R)
                nc.sync.dma_start(
                    out=WqT_sb,
                    in_=wqt_in[:, :].rearrange("(pr p) c -> p pr c", p=P))
                WqpT_sb = wqp.tile([P, NPAIR, C], F32R)
                for i in range(NPAIR):
                    for ch in range(2):
                        ps_w = wqps.tile([P, 512], F32, tag="wq")
                        nc.tensor.matmul(
                            out=ps_w,
                            lhsT=blockdiag[i],
                            rhs=WqT_sb[:, i, ch * 512:(ch + 1) * 512],
                            start=True, stop=True)
                        # fold softmax 1/rowsum (per gd partition) into Wq'
                        nc.scalar.activation(
                            out=WqpT_sb[:, i, ch * 512:(ch + 1) * 512],
                            in_=ps_w,
                            func=mybir.ActivationFunctionType.Identity,
                            bias=0.0, scale=recip[:, i:i + 1])

                with tc.tile_pool(name="wpp", bufs=1) as wpp, \
                     tc.tile_pool(name="mps", bufs=2, space="PSUM") as mps:
                    Wp_sb = wpp.tile([P, NPAIR, C], F32R)
                    nc.sync.dma_start(
                        out=Wp_sb,
                        in_=wp_in[:, :].rearrange("(pr p) c -> p pr c", p=P))
                    for cch in range(CCH):
                        ps_m = mps.tile([P, C], F32, tag="mp")
                        for ch in range(2):
                            for i in range(NPAIR):
                                nc.tensor.matmul(
                                    out=ps_m[:, ch * 512:(ch + 1) * 512],
                                    lhsT=WqpT_sb[:, i, cch * P:(cch + 1) * P],
                                    rhs=Wp_sb[:, i, ch * 512:(ch + 1) * 512],
                                    start=(i == 0), stop=(i == NPAIR - 1),
                                )
                        copy_psum(Msb[:, cch, :], ps_m, cch % 2 == 1)
                        # h = x @ (M + I): add identity on the diagonal block
                        nc.vector.tensor_add(
                            out=Msb[:, cch, cch * P:(cch + 1) * P],
                            in0=Msb[:, cch, cch * P:(cch + 1) * P],
                            in1=ident_r)

            # ---------------- y = x @ (M+I), LayerNorm off PSUM ----------
            with tc.tile_pool(name="xtp", bufs=2) as xtp, \
                 tc.tile_pool(name="outp", bufs=3) as outp, \
                 tc.tile_pool(name="lnp", bufs=4) as lnp, \
                 tc.tile_pool(name="yps", bufs=4, space="PSUM") as yps:
                for chunk in range(8):
                    n0 = chunk * 512
                    xt_sb = xtp.tile([P, CCH, 512], F32R, tag="xt")
                    nc.sync.dma_start(
                        out=xt_sb,
                        in_=xt_in[:, n0:n0 + 512].rearrange(
                            "(cc p) n -> p cc n", p=P))
                    for ns in range(4):
                        pa = yps.tile([P, 512], F32, tag="ya")
                        pb = yps.tile([P, 512], F32, tag="yb")
                        for half, pt in ((0, pa), (1, pb)):
                            for k in range(CCH):
                                nc.tensor.matmul(
                                    out=pt,
                                    lhsT=xt_sb[:, k, ns * P:(ns + 1) * P],
                                    rhs=Msb[:, k, half * 512:(half + 1) * 512],
                                    start=(k == 0), stop=(k == CCH - 1),
                                )
                            if need_bias:
                                nc.tensor.matmul(
                                    out=pt, lhsT=ones_r,
                                    rhs=bias_sb[:, half * 512:(half + 1) * 512],
                                    start=False, stop=True)
                        stats = lnp.tile([P, 2, 6], F32, tag="stats")
                        nc.vector.bn_stats(out=stats[:, 0, :], in_=pa)
                        nc.vector.bn_stats(out=stats[:, 1, :], in_=pb)
                        mv = lnp.tile([P, 2], F32, tag="mv")
                        nc.vector.bn_aggr(out=mv, in_=stats)
                        rstd = lnp.tile([P, 1], F32, tag="rstd")
                        nc.scalar.activation(
                            out=rstd, in_=mv[:, 1:2],
                            func=mybir.ActivationFunctionType.Sqrt,
                            bias=eps_sb, scale=1.0)
                        nc.vector.reciprocal(out=rstd, in_=rstd)
                        nmur = lnp.tile([P, 1], F32, tag="nmur")
                        nc.vector.tensor_mul(out=nmur, in0=mv[:, 0:1], in1=rstd)
                        nc.scalar.mul(out=nmur, in_=nmur, mul=-1.0)
                        o_sb = outp.tile([P, C], F32, tag="osb")
                        nc.vector.tensor_scalar(
                            out=o_sb[:, 0:512], in0=pa,
                            scalar1=mv[:, 0:1], scalar2=rstd,
                            op0=mybir.AluOpType.subtract,
                            op1=mybir.AluOpType.mult)
                        nc.scalar.activation(
                            out=o_sb[:, 512:C], in_=pb,
                            func=mybir.ActivationFunctionType.Identity,
                            bias=nmur, scale=rstd)
                        if need_gamma:
                            nc.vector.tensor_mul(out=o_sb, in0=o_sb, in1=gamma_sb)
                        if need_beta:
                            nc.vector.tensor_add(out=o_sb, in0=o_sb, in1=beta_sb)
                        nc.sync.dma_start(
                            out=out_d[n0 + ns * P: n0 + (ns + 1) * P, :],
                            in_=o_sb)

    nc.compile()
    return nc


def _get_nc(need_bias, need_gamma, need_beta):
    key = (need_bias, need_gamma, need_beta)
    if key not in _cache:
        _cache[key] = _build(*key)
    return _cache[key]


def kernel(x, Wqkv, Wproj, bproj, gamma, beta):
    x = np.asarray(x, dtype=np.float32)
    Wqkv = np.asarray(Wqkv, dtype=np.float32)
    Wproj = np.asarray(Wproj, dtype=np.float32)
    bproj = np.asarray(bproj, dtype=np.float32)
    gamma = np.asarray(gamma, dtype=np.float32)
    beta = np.asarray(beta, dtype=np.float32)

    need_bias = bool(np.any(bproj))
    need_gamma = not bool(np.all(gamma == 1.0))
    need_beta = bool(np.any(beta))
    nc = _get_nc(need_bias, need_gamma, need_beta)

    wq = Wqkv[:, 0:C]
    wk = Wqkv[:, C:2 * C] * np.float32(SCALE)
    wv = Wqkv[:, 2 * C:3 * C]
    wqt = np.ascontiguousarray(wq.T)
    wk = np.ascontiguousarray(wk)
    wv = np.ascontiguousarray(wv)
    wp = np.ascontiguousarray(Wproj)

    in_maps = []
    for b in range(B):
        m = {
            "x": np.ascontiguousarray(x[b]),
            "xt": np.ascontiguousarray(x[b].T),
            "wv": wv, "wk": wk, "wqt": wqt, "wp": wp,
        }
        if need_bias:
            m["bias"] = bproj.reshape(1, C)
        if need_gamma:
            m["gamma"] = gamma.reshape(1, C)
        if need_beta:
            m["beta"] = beta.reshape(1, C)
        in_maps.append(m)

    res = run_bass_kernel_spmd(nc, in_maps, list(range(B)))
    return np.stack([res.results[b]["out"] for b in range(B)], axis=0)


if __name__ == "__main__":
    rng = np.random.default_rng(0)
    x = rng.standard_normal((B, N, C)).astype(np.float32)
    Wqkv = (rng.standard_normal((C, 3 * C)) * C ** -0.5).astype(np.float32)
    Wproj = (rng.standard_normal((C, C)) * C ** -0.5).astype(np.float32)
    out = kernel(x, Wqkv, Wproj, np.zeros(C, np.float32),
                 np.ones(C, np.float32), np.zeros(C, np.float32))
    print("out", out.shape, out.dtype, np.abs(out).max())
